# revision 5
# baseline (speedup 1.0000x reference)
"""Causal self-attention (QKV proj + RoPE + causal softmax attention + out proj)
for Trainium2, distributed over 8 NeuronCores.

Sharding: 4 batches x 2 head-groups (tensor parallel over heads within a batch).
Each core computes, for its (batch b, head-group g of 8 heads):
  - qkv = x[b] @ w_qkv[:, cols(g)]   (feature-major via lhsT = w chunks)
  - RoPE on q, k
  - causal softmax attention for its 8 heads (transposed-scores layout)
  - out_partial = att @ w_proj[rows(g), :]
Host gathers: out[b] = partial(b,0) + partial(b,1)  (the 2-way TP all-reduce),
and reassembles k, v from per-core feature-major slices.

All matmuls run in float32r (fp32 data, hardware rounds operands to 11 mantissa
bits, ~bf16 throughput at N>=512).
"""

import math

import numpy as np

import concourse.bass as bass
import concourse.tile as tile
import concourse.mybir as mybir
from concourse import bacc
from concourse.bass_utils import run_bass_kernel_spmd

F32 = mybir.dt.float32
F32R = mybir.dt.float32r
BF16 = mybir.dt.bfloat16
AF = mybir.ActivationFunctionType

B, T, C = 4, 2048, 2048
N_HEAD = 16
D = C // N_HEAD          # 128
HALF = D // 2            # 64
HPG = 8                  # heads per group (2 groups)
ROPE_BASE = 10000.0
ISQRT_D = 1.0 / math.sqrt(D)

NCTILE = C // 128        # 16 contraction tiles
NQB = T // 512           # 4 query blocks of 512
NKB = T // 128           # 16 key blocks of 128


def _build_program():
    nc = bacc.Bacc("TRN2", target_bir_lowering=False, debug=False, num_devices=8)

    # per-core inputs
    xT_d = nc.dram_tensor("xT", [NCTILE, 128, T], F32R, kind="ExternalInput").ap()
    wqkv_d = nc.dram_tensor("wqkv", [C, 3 * HPG * D], F32R, kind="ExternalInput").ap()
    wproj_d = nc.dram_tensor("wproj", [HPG * D, C], F32R, kind="ExternalInput").ap()
    cos_d = nc.dram_tensor("cosd", [HALF, T], F32, kind="ExternalInput").ap()
    sin_d = nc.dram_tensor("sind", [HALF, T], F32, kind="ExternalInput").ap()
    mask_d = nc.dram_tensor("trimask", [128, 128], BF16, kind="ExternalInput").ap()

    # per-core outputs
    out_d = nc.dram_tensor("out_part", [T, C], F32, kind="ExternalOutput").ap()
    k_out_d = nc.dram_tensor("k_out", [HPG, D, T], F32, kind="ExternalOutput").ap()
    v_out_d = nc.dram_tensor("v_out", [HPG, D, T], F32, kind="ExternalOutput").ap()

    with tile.TileContext(nc) as tc:
        with tc.tile_pool(name="dram", bufs=1, space="DRAM") as dpool, \
             tc.tile_pool(name="vall", bufs=1) as vallp, \
             tc.tile_pool(name="const", bufs=1) as constp:

            q_scr = dpool.tile([HPG, 128, T], F32R, tag="q_scr")
            k_scr = dpool.tile([HPG, 128, T], F32R, tag="k_scr")
            v_all = vallp.tile([128, HPG, T], F32R, tag="v_all")

            mask_sb = constp.tile([128, 128], BF16, tag="mask")
            nc.sync.dma_start(out=mask_sb[:], in_=mask_d)
            ones128_f = constp.tile([128, 1], F32, tag="ones128f")
            nc.vector.memset(ones128_f[:], 1.0)
            ones128 = constp.tile([128, 1], F32R, tag="ones128")
            nc.vector.tensor_copy(ones128[:], ones128_f[:])
            ones1_f = constp.tile([1, 128], F32, tag="ones1f")
            nc.vector.memset(ones1_f[:], 1.0)
            ones1 = constp.tile([1, 128], F32R, tag="ones1")
            nc.vector.tensor_copy(ones1[:], ones1_f[:])
            ident_f = constp.tile([128, 128], F32, tag="ident_f")
            from concourse.masks import make_identity
            make_identity(nc, ident_f[:])
            ident = constp.tile([128, 128], F32R, tag="ident")
            nc.vector.tensor_copy(ident[:], ident_f[:])

            # ---------------- Phase 1: QKV projection + RoPE ----------------
            with tc.tile_pool(name="xh", bufs=1) as xp, \
                 tc.tile_pool(name="wq", bufs=3) as wp, \
                 tc.tile_pool(name="qkvps", bufs=3, space="PSUM") as qkvps, \
                 tc.tile_pool(name="stag", bufs=2) as stp, \
                 tc.tile_pool(name="rope", bufs=2) as rp, \
                 tc.tile_pool(name="cs", bufs=1) as csp:

                cos_sb = csp.tile([HALF, T], F32, tag="cos")
                nc.sync.dma_start(out=cos_sb[:], in_=cos_d)
                sin_sb = csp.tile([HALF, T], F32, tag="sin")
                nc.sync.dma_start(out=sin_sb[:], in_=sin_d)

                def rope(ps, tb, out_tile):
                    ts_ = slice(tb * 512, (tb + 1) * 512)
                    t1 = rp.tile([HALF, 512], F32, tag="t1")
                    nc.vector.tensor_mul(t1[:], ps[0:HALF, :], cos_sb[:, ts_])
                    t2 = rp.tile([HALF, 512], F32, tag="t2")
                    nc.vector.tensor_mul(t2[:], ps[HALF:128, :], sin_sb[:, ts_])
                    nc.vector.tensor_sub(out_tile[0:HALF, :], t1[:], t2[:])
                    t3 = rp.tile([HALF, 512], F32, tag="t3")
                    nc.vector.tensor_mul(t3[:], ps[0:HALF, :], sin_sb[:, ts_])
                    t4 = rp.tile([HALF, 512], F32, tag="t4")
                    nc.vector.tensor_mul(t4[:], ps[HALF:128, :], cos_sb[:, ts_])
                    nc.vector.tensor_add(out_tile[HALF:128, :], t3[:], t4[:])

                for half in range(2):
                    xh = xp.tile([128, NCTILE, 1024], F32R, tag="xh")
                    for ci in range(NCTILE):
                        nc.sync.dma_start(
                            out=xh[:, ci, :],
                            in_=xT_d[ci, :, half * 1024:(half + 1) * 1024])
                    for fb in range(3 * HPG):
                        w_t = wp.tile([128, NCTILE, 128], F32R, tag="w")
                        for ci in range(NCTILE):
                            nc.sync.dma_start(
                                out=w_t[:, ci, :],
                                in_=wqkv_d[ci * 128:(ci + 1) * 128,
                                           fb * 128:(fb + 1) * 128])
                        for tbl in range(2):
                            tb = half * 2 + tbl
                            ps = qkvps.tile([128, 512], F32, tag="qkvps")
                            for ci in range(NCTILE):
                                nc.tensor.matmul(
                                    ps[:], w_t[:, ci, :],
                                    xh[:, ci, tbl * 512:(tbl + 1) * 512],
                                    start=(ci == 0), stop=(ci == NCTILE - 1))
                            ts_ = slice(tb * 512, (tb + 1) * 512)
                            if fb < HPG:           # q
                                h = fb
                                qf = stp.tile([128, 512], F32R, tag="qstag")
                                rope(ps, tb, qf)
                                nc.sync.dma_start(out=q_scr[h, :, ts_], in_=qf[:])
                            elif fb < 2 * HPG:     # k
                                h = fb - HPG
                                kf = stp.tile([128, 512], F32, tag="kstag")
                                rope(ps, tb, kf)
                                nc.sync.dma_start(out=k_out_d[h, :, ts_], in_=kf[:])
                                kfr = stp.tile([128, 512], F32R, tag="kstagr")
                                nc.scalar.copy(kfr[:], kf[:])
                                nc.sync.dma_start(out=k_scr[h, :, ts_], in_=kfr[:])
                            else:                  # v
                                h = fb - 2 * HPG
                                vf = stp.tile([128, 512], F32, tag="vstag")
                                nc.scalar.copy(vf[:], ps[:])
                                nc.sync.dma_start(out=v_out_d[h, :, ts_], in_=vf[:])
                                nc.vector.tensor_copy(v_all[:, h, ts_], ps[:])

            # ---------------- Phase 2+3: attention, then projection ----------
            with tc.tile_pool(name="att", bufs=1) as attp:
                att_all = attp.tile([128, HPG, T], F32R, tag="att_all")

                with tc.tile_pool(name="qh", bufs=2) as qhp, \
                     tc.tile_pool(name="kh", bufs=2) as khp, \
                     tc.tile_pool(name="vtok", bufs=2) as vtp, \
                     tc.tile_pool(name="vtps", bufs=1, space="PSUM") as vtps, \
                     tc.tile_pool(name="sps", bufs=3, space="PSUM") as sps, \
                     tc.tile_pool(name="sumps", bufs=1, space="PSUM") as sumps, \
                     tc.tile_pool(name="avps", bufs=2, space="PSUM") as avps, \
                     tc.tile_pool(name="bps", bufs=1, space="PSUM") as bps, \
                     tc.tile_pool(name="probs", bufs=6) as prp, \
                     tc.tile_pool(name="bc", bufs=2) as bcp, \
                     tc.tile_pool(name="rcp", bufs=2) as rcp:

                    for h in range(HPG):
                        q_t = qhp.tile([128, T], F32R, tag="qh")
                        nc.sync.dma_start(out=q_t[:], in_=q_scr[h, :, :])
                        k_t = khp.tile([128, T], F32R, tag="kh")
                        nc.sync.dma_start(out=k_t[:], in_=k_scr[h, :, :])
                        vtok = vtp.tile([128, NKB, 128], F32R, tag="vtok")
                        for tbk in range(NKB):
                            pvt = vtps.tile([128, 128], F32R, tag="vtps")
                            nc.tensor.transpose(
                                pvt[:], v_all[:, h, tbk * 128:(tbk + 1) * 128],
                                ident[:])
                            nc.vector.tensor_copy(vtok[:, tbk, :], pvt[:])

                        for qi in range(NQB):
                            nkb = 4 * qi + 4
                            qs = slice(qi * 512, (qi + 1) * 512)
                            ps_sum = sumps.tile([1, 512], F32, tag="ps_sum")
                            ps_o = avps.tile([128, 512], F32, tag="ps_o")
                            pending = []

                            def sum_av(j, pr, left, *, _sum=ps_sum, _o=ps_o,
                                       _nkb=nkb):
                                cs_ = slice(left, 512)
                                nc.tensor.matmul(
                                    _sum[0:1, cs_], ones128[:], pr[:, cs_],
                                    start=(j == 0), stop=(j == _nkb - 1))
                                nc.tensor.matmul(
                                    _o[:, cs_], vtok[:, j, :], pr[:, cs_],
                                    start=(j == 0), stop=(j == _nkb - 1))

                            for kb in range(nkb):
                                ps_s = sps.tile([128, 512], F32, tag="ps_s")
                                nc.tensor.matmul(
                                    ps_s[:], k_t[:, kb * 128:(kb + 1) * 128],
                                    q_t[:, qs], start=True, stop=True)
                                m = kb - 4 * qi
                                left = max(m, 0) * 128
                                pr = prp.tile([128, 512], F32R, tag="probs")
                                nc.scalar.activation(
                                    pr[:, left:], ps_s[:, left:], AF.Exp,
                                    scale=ISQRT_D)
                                if m >= 0:
                                    dg = slice(left, left + 128)
                                    nc.vector.tensor_mul(
                                        pr[:, dg], pr[:, dg], mask_sb[:])
                                pending.append((kb, pr, left))
                                if len(pending) > 2:
                                    sum_av(*pending.pop(0))
                            while pending:
                                sum_av(*pending.pop(0))

                            recip = rcp.tile([1, 512], F32, tag="recip")
                            nc.vector.reciprocal(recip[:], ps_sum[:])
                            recipr = rcp.tile([1, 512], F32R, tag="recipr")
                            nc.scalar.copy(recipr[:], recip[:])
                            ps_b = bps.tile([128, 512], F32, tag="ps_b")
                            nc.tensor.matmul(ps_b[:], ones1[:], recipr[:],
                                             start=True, stop=True)
                            bc = bcp.tile([128, 512], F32R, tag="bc")
                            nc.scalar.copy(bc[:], ps_b[:])
                            nc.vector.tensor_mul(att_all[:, h, qs], ps_o[:], bc[:])

                # ---------------- Phase 3: output projection ----------------
                with tc.tile_pool(name="wp", bufs=2) as wpp, \
                     tc.tile_pool(name="pps", bufs=3, space="PSUM") as pps, \
                     tc.tile_pool(name="ostag", bufs=3) as osp:
                    for cb in range(4):
                        wp_t = wpp.tile([128, HPG, 512], F32R, tag="wp")
                        for hh in range(HPG):
                            nc.sync.dma_start(
                                out=wp_t[:, hh, :],
                                in_=wproj_d[hh * 128:(hh + 1) * 128,
                                            cb * 512:(cb + 1) * 512])
                        for tb in range(NKB):
                            ps_p = pps.tile([128, 512], F32, tag="ps_p")
                            for hh in range(HPG):
                                nc.tensor.matmul(
                                    ps_p[:],
                                    att_all[:, hh, tb * 128:(tb + 1) * 128],
                                    wp_t[:, hh, :],
                                    start=(hh == 0), stop=(hh == HPG - 1))
                            o_sb = osp.tile([128, 512], F32, tag="ostag")
                            nc.vector.tensor_copy(o_sb[:], ps_p[:])
                            nc.sync.dma_start(
                                out=out_d[tb * 128:(tb + 1) * 128,
                                          cb * 512:(cb + 1) * 512],
                                in_=o_sb[:])

    nc.compile()
    return nc


_NC_CACHE = None


def _get_program():
    global _NC_CACHE
    if _NC_CACHE is None:
        _NC_CACHE = _build_program()
    return _NC_CACHE


def _host_prep(x, w_qkv, w_proj):
    """Build the 8 per-core input maps (core = b*2 + g)."""
    x = np.ascontiguousarray(np.asarray(x, dtype=np.float32))
    w_qkv = np.asarray(w_qkv, dtype=np.float32)
    w_proj = np.asarray(w_proj, dtype=np.float32)

    # rope tables, feature-major [HALF, T]
    inv_freq = 1.0 / (ROPE_BASE ** (np.arange(HALF, dtype=np.float32) / HALF))
    pos = np.arange(T, dtype=np.float32)
    freqs = pos[:, None] * inv_freq[None, :]          # [T, HALF]
    cos_d = np.ascontiguousarray(np.cos(freqs).T.astype(np.float32))
    sin_d = np.ascontiguousarray(np.sin(freqs).T.astype(np.float32))

    import ml_dtypes
    tri = np.triu(np.ones((128, 128), dtype=np.float32))  # [k,q]: 1 if k<=q
    tri = tri.astype(ml_dtypes.bfloat16)

    in_maps = []
    for core in range(8):
        b, g = divmod(core, 2)
        xT = np.ascontiguousarray(x[b].T).reshape(NCTILE, 128, T)
        cols = []
        for part in range(3):                          # q, k, v column groups
            base = part * C + g * HPG * D
            cols.append(w_qkv[:, base:base + HPG * D])
        wq = np.ascontiguousarray(np.concatenate(cols, axis=1))
        wpj = np.ascontiguousarray(w_proj[g * HPG * D:(g + 1) * HPG * D, :])
        in_maps.append({
            "xT": xT, "wqkv": wq, "wproj": wpj,
            "cosd": cos_d, "sind": sin_d, "trimask": tri,
        })
    return in_maps


def kernel(x, w_qkv, w_proj):
    nc = _get_program()
    in_maps = _host_prep(x, w_qkv, w_proj)
    res = run_bass_kernel_spmd(nc, in_maps, list(range(8)))

    out = np.empty((B, T, C), dtype=np.float32)
    k = np.empty((B, N_HEAD, T, D), dtype=np.float32)
    v = np.empty((B, N_HEAD, T, D), dtype=np.float32)
    for b in range(B):
        r0 = res.results[2 * b]
        r1 = res.results[2 * b + 1]
        out[b] = r0["out_part"] + r1["out_part"]
        for g, r in ((0, r0), (1, r1)):
            hs = slice(g * HPG, (g + 1) * HPG)
            k[b, hs] = np.transpose(r["k_out"], (0, 2, 1))
            v[b, hs] = np.transpose(r["v_out"], (0, 2, 1))
    return out, k, v


# revision 7
# speedup vs baseline: 1.3566x; 1.3566x over previous
"""Causal self-attention (QKV proj + RoPE + causal softmax attention + out proj)
for Trainium2, distributed over 8 NeuronCores.

Sharding: 4 batches x 2 head-groups (tensor parallel over heads within a batch).
Each core computes, for its (batch b, head-group g of 8 heads):
  - qkv = x[b] @ w_qkv[:, cols(g)]   (feature-major via lhsT = w chunks)
  - RoPE on q, k
  - causal softmax attention for its 8 heads (transposed-scores layout)
  - out_partial = att @ w_proj[rows(g), :]
Host gathers: out[b] = partial(b,0) + partial(b,1)  (the 2-way TP all-reduce),
and reassembles k, v from per-core feature-major slices.

All matmuls run in float32r (fp32 data, hardware rounds operands to 11 mantissa
bits, ~bf16 throughput at N>=512).
"""

import math

import numpy as np

import concourse.bass as bass
import concourse.tile as tile
import concourse.mybir as mybir
from concourse import bacc
from concourse.bass_utils import run_bass_kernel_spmd

F32 = mybir.dt.float32
F32R = mybir.dt.float32r
BF16 = mybir.dt.bfloat16
AF = mybir.ActivationFunctionType

B, T, C = 4, 2048, 2048
N_HEAD = 16
D = C // N_HEAD          # 128
HALF = D // 2            # 64
HPG = 8                  # heads per group (2 groups)
ROPE_BASE = 10000.0
ISQRT_D = 1.0 / math.sqrt(D)

NCTILE = C // 128        # 16 contraction tiles
NQB = T // 512           # 4 query blocks of 512
NKB = T // 128           # 16 key blocks of 128


def _build_program():
    nc = bacc.Bacc("TRN2", target_bir_lowering=False, debug=False, num_devices=8)

    # per-core inputs
    xT_d = nc.dram_tensor("xT", [NCTILE, 128, T], F32R, kind="ExternalInput").ap()
    wqkv_d = nc.dram_tensor("wqkv", [C, 3 * HPG * D], F32R, kind="ExternalInput").ap()
    wproj_d = nc.dram_tensor("wproj", [HPG * D, C], F32R, kind="ExternalInput").ap()
    cos_d = nc.dram_tensor("cosd", [HALF, T], F32, kind="ExternalInput").ap()
    sin_d = nc.dram_tensor("sind", [HALF, T], F32, kind="ExternalInput").ap()
    mask_d = nc.dram_tensor("trimask", [128, 128], BF16, kind="ExternalInput").ap()

    # per-core outputs
    out_d = nc.dram_tensor("out_part", [T, C], F32, kind="ExternalOutput").ap()
    k_out_d = nc.dram_tensor("k_out", [HPG, D, T], F32, kind="ExternalOutput").ap()
    v_out_d = nc.dram_tensor("v_out", [HPG, D, T], F32, kind="ExternalOutput").ap()

    wqkv_r = wqkv_d.rearrange("(c p) f -> p c f", p=128)    # [128, 16, 3072]
    xT_r = xT_d.rearrange("c p t -> p c t")                 # [128, 16, 2048]
    wproj_r = wproj_d.rearrange("(h p) c -> p h c", p=128)  # [128, 8, 2048]

    with tile.TileContext(nc) as tc:
        with tc.tile_pool(name="dram", bufs=1, space="DRAM") as dpool, \
             tc.tile_pool(name="const", bufs=1) as constp:

            q_scr = dpool.tile([HPG, 128, T], F32R, tag="q_scr")
            k_scr = dpool.tile([HPG, 128, T], F32R, tag="k_scr")
            v_scr = dpool.tile([HPG, 128, T], F32R, tag="v_scr")

            mask_sb = constp.tile([128, 128], BF16, tag="mask")
            nc.sync.dma_start(out=mask_sb[:], in_=mask_d)
            ones128_f = constp.tile([128, 1], F32, tag="ones128f")
            nc.vector.memset(ones128_f[:], 1.0)
            ones128 = constp.tile([128, 1], F32R, tag="ones128")
            nc.vector.tensor_copy(ones128[:], ones128_f[:])
            ones1_f = constp.tile([1, 128], F32, tag="ones1f")
            nc.vector.memset(ones1_f[:], 1.0)
            ones1 = constp.tile([1, 128], F32R, tag="ones1")
            nc.vector.tensor_copy(ones1[:], ones1_f[:])
            ident_f = constp.tile([128, 128], F32, tag="ident_f")
            from concourse.masks import make_identity
            make_identity(nc, ident_f[:])
            ident = constp.tile([128, 128], F32R, tag="ident")
            nc.vector.tensor_copy(ident[:], ident_f[:])

            # ---------------- Phase 1: QKV projection + RoPE ----------------
            with tc.tile_pool(name="xh", bufs=1) as xp, \
                 tc.tile_pool(name="wq", bufs=3) as wp, \
                 tc.tile_pool(name="qkvps", bufs=8, space="PSUM") as qkvps, \
                 tc.tile_pool(name="stag", bufs=2) as stp, \
                 tc.tile_pool(name="rope", bufs=2) as rp, \
                 tc.tile_pool(name="cs", bufs=1) as csp:

                cos_sb = csp.tile([HALF, T], F32, tag="cos")
                nc.sync.dma_start(out=cos_sb[:], in_=cos_d)
                sin_sb = csp.tile([HALF, T], F32, tag="sin")
                nc.sync.dma_start(out=sin_sb[:], in_=sin_d)

                xh = xp.tile([128, NCTILE, T], F32R, tag="xh")
                nc.sync.dma_start(out=xh[:], in_=xT_r)

                def rope(ps, tb, out_tile):
                    ts_ = slice(tb * 512, (tb + 1) * 512)
                    t1 = rp.tile([HALF, 512], F32, tag="t1")
                    nc.vector.tensor_mul(t1[:], ps[0:HALF, :], cos_sb[:, ts_])
                    t2 = rp.tile([HALF, 512], F32, tag="t2")
                    nc.vector.tensor_mul(t2[:], ps[HALF:128, :], sin_sb[:, ts_])
                    nc.vector.tensor_sub(out_tile[0:HALF, :], t1[:], t2[:])
                    t3 = rp.tile([HALF, 512], F32, tag="t3")
                    nc.vector.tensor_mul(t3[:], ps[0:HALF, :], sin_sb[:, ts_])
                    t4 = rp.tile([HALF, 512], F32, tag="t4")
                    nc.vector.tensor_mul(t4[:], ps[HALF:128, :], cos_sb[:, ts_])
                    nc.vector.tensor_add(out_tile[HALF:128, :], t3[:], t4[:])

                for fb in range(3 * HPG):
                    w_t = wp.tile([128, NCTILE, 128], F32R, tag="w")
                    nc.sync.dma_start(
                        out=w_t[:], in_=wqkv_r[:, :, fb * 128:(fb + 1) * 128])
                    pss = [qkvps.tile([128, 512], F32, tag="qkvps",
                                      name=f"qkvps_{fb}_{i}")
                           for i in range(NQB)]
                    for ci in range(NCTILE):
                        for tb in range(NQB):
                            nc.tensor.matmul(
                                pss[tb][:], w_t[:, ci, :],
                                xh[:, ci, tb * 512:(tb + 1) * 512],
                                start=(ci == 0), stop=(ci == NCTILE - 1))
                    for tb in range(NQB):
                        ps = pss[tb]
                        ts_ = slice(tb * 512, (tb + 1) * 512)
                        if fb < HPG:           # q
                            h = fb
                            qf = stp.tile([128, 512], F32R, tag="qstag")
                            rope(ps, tb, qf)
                            nc.sync.dma_start(out=q_scr[h, :, ts_], in_=qf[:])
                        elif fb < 2 * HPG:     # k
                            h = fb - HPG
                            kf = stp.tile([128, 512], F32, tag="kstag")
                            rope(ps, tb, kf)
                            nc.sync.dma_start(out=k_out_d[h, :, ts_], in_=kf[:])
                            nc.sync.dma_start(out=k_scr[h, :, ts_],
                                              in_=kf[:].bitcast(F32R))
                        else:                  # v
                            h = fb - 2 * HPG
                            vf = stp.tile([128, 512], F32, tag="vstag")
                            nc.vector.tensor_copy(vf[:], ps[:])
                            nc.sync.dma_start(out=v_out_d[h, :, ts_], in_=vf[:])
                            nc.sync.dma_start(out=v_scr[h, :, ts_],
                                              in_=vf[:].bitcast(F32R))

            # ---------------- Phase 2+3: attention, then projection ----------
            with tc.tile_pool(name="att", bufs=1) as attp:
                att_all = attp.tile([128, HPG, T], F32R, tag="att_all")

                with tc.tile_pool(name="qh", bufs=2) as qhp, \
                     tc.tile_pool(name="kh", bufs=2) as khp, \
                     tc.tile_pool(name="vh", bufs=2) as vhp, \
                     tc.tile_pool(name="vtok", bufs=2) as vtp, \
                     tc.tile_pool(name="vtps", bufs=1, space="PSUM") as vtps, \
                     tc.tile_pool(name="sps", bufs=3, space="PSUM") as sps, \
                     tc.tile_pool(name="sumps", bufs=1, space="PSUM") as sumps, \
                     tc.tile_pool(name="avps", bufs=2, space="PSUM") as avps, \
                     tc.tile_pool(name="bps", bufs=1, space="PSUM") as bps, \
                     tc.tile_pool(name="probs", bufs=6) as prp, \
                     tc.tile_pool(name="bc", bufs=2) as bcp, \
                     tc.tile_pool(name="rcp", bufs=2) as rcp:

                    for h in range(HPG):
                        q_t = qhp.tile([128, T], F32R, tag="qh")
                        nc.sync.dma_start(out=q_t[:], in_=q_scr[h, :, :])
                        k_t = khp.tile([128, T], F32R, tag="kh")
                        nc.sync.dma_start(out=k_t[:], in_=k_scr[h, :, :])
                        v_h = vhp.tile([128, T], F32R, tag="vh")
                        nc.sync.dma_start(out=v_h[:], in_=v_scr[h, :, :])
                        vtok = vtp.tile([128, NKB, 128], F32R, tag="vtok")
                        for tbk in range(NKB):
                            pvt = vtps.tile([128, 128], F32R, tag="vtps")
                            nc.tensor.transpose(
                                pvt[:], v_h[:, tbk * 128:(tbk + 1) * 128],
                                ident[:])
                            nc.vector.tensor_copy(vtok[:, tbk, :], pvt[:])

                        for qi in range(NQB):
                            nkb = 4 * qi + 4
                            qs = slice(qi * 512, (qi + 1) * 512)
                            ps_sum = sumps.tile([1, 512], F32, tag="ps_sum")
                            ps_o = avps.tile([128, 512], F32, tag="ps_o")
                            pending = []

                            def sum_av(j, pr, left, *, _sum=ps_sum, _o=ps_o,
                                       _nkb=nkb):
                                cs_ = slice(left, 512)
                                nc.tensor.matmul(
                                    _sum[0:1, cs_], ones128[:], pr[:, cs_],
                                    start=(j == 0), stop=(j == _nkb - 1))
                                nc.tensor.matmul(
                                    _o[:, cs_], vtok[:, j, :], pr[:, cs_],
                                    start=(j == 0), stop=(j == _nkb - 1))

                            for kb in range(nkb):
                                ps_s = sps.tile([128, 512], F32, tag="ps_s")
                                nc.tensor.matmul(
                                    ps_s[:], k_t[:, kb * 128:(kb + 1) * 128],
                                    q_t[:, qs], start=True, stop=True)
                                m = kb - 4 * qi
                                left = max(m, 0) * 128
                                pr = prp.tile([128, 512], F32R, tag="probs")
                                nc.scalar.activation(
                                    pr[:, left:], ps_s[:, left:], AF.Exp,
                                    scale=ISQRT_D)
                                if m >= 0:
                                    dg = slice(left, left + 128)
                                    nc.vector.tensor_mul(
                                        pr[:, dg], pr[:, dg], mask_sb[:])
                                pending.append((kb, pr, left))
                                if len(pending) > 2:
                                    sum_av(*pending.pop(0))
                            while pending:
                                sum_av(*pending.pop(0))

                            recip = rcp.tile([1, 512], F32, tag="recip")
                            nc.vector.reciprocal_approx_fast(
                                out=recip[:], in_=ps_sum[:])
                            recipr = rcp.tile([1, 512], F32R, tag="recipr")
                            nc.vector.tensor_copy(recipr[:], recip[:])
                            ps_b = bps.tile([128, 512], F32, tag="ps_b")
                            nc.tensor.matmul(ps_b[:], ones1[:], recipr[:],
                                             start=True, stop=True)
                            bc = bcp.tile([128, 512], F32R, tag="bc")
                            nc.scalar.copy(bc[:], ps_b[:])
                            nc.vector.tensor_mul(att_all[:, h, qs], ps_o[:], bc[:])

                # ---------------- Phase 3: output projection ----------------
                with tc.tile_pool(name="wp", bufs=2) as wpp, \
                     tc.tile_pool(name="pps", bufs=3, space="PSUM") as pps, \
                     tc.tile_pool(name="ostag", bufs=3) as osp:
                    for cb in range(4):
                        wp_t = wpp.tile([128, HPG, 512], F32R, tag="wp")
                        nc.sync.dma_start(
                            out=wp_t[:],
                            in_=wproj_r[:, :, cb * 512:(cb + 1) * 512])
                        for tb in range(NKB):
                            ps_p = pps.tile([128, 512], F32, tag="ps_p")
                            for hh in range(HPG):
                                nc.tensor.matmul(
                                    ps_p[:],
                                    att_all[:, hh, tb * 128:(tb + 1) * 128],
                                    wp_t[:, hh, :],
                                    start=(hh == 0), stop=(hh == HPG - 1))
                            o_sb = osp.tile([128, 512], F32, tag="ostag")
                            nc.vector.tensor_copy(o_sb[:], ps_p[:])
                            nc.sync.dma_start(
                                out=out_d[tb * 128:(tb + 1) * 128,
                                          cb * 512:(cb + 1) * 512],
                                in_=o_sb[:])

    nc.compile()
    return nc


_NC_CACHE = None


def _get_program():
    global _NC_CACHE
    if _NC_CACHE is None:
        _NC_CACHE = _build_program()
    return _NC_CACHE


def _host_prep(x, w_qkv, w_proj):
    """Build the 8 per-core input maps (core = b*2 + g)."""
    x = np.ascontiguousarray(np.asarray(x, dtype=np.float32))
    w_qkv = np.asarray(w_qkv, dtype=np.float32)
    w_proj = np.asarray(w_proj, dtype=np.float32)

    # rope tables, feature-major [HALF, T]
    inv_freq = 1.0 / (ROPE_BASE ** (np.arange(HALF, dtype=np.float32) / HALF))
    pos = np.arange(T, dtype=np.float32)
    freqs = pos[:, None] * inv_freq[None, :]          # [T, HALF]
    cos_d = np.ascontiguousarray(np.cos(freqs).T.astype(np.float32))
    sin_d = np.ascontiguousarray(np.sin(freqs).T.astype(np.float32))

    import ml_dtypes
    tri = np.triu(np.ones((128, 128), dtype=np.float32))  # [k,q]: 1 if k<=q
    tri = tri.astype(ml_dtypes.bfloat16)

    in_maps = []
    for core in range(8):
        b, g = divmod(core, 2)
        xT = np.ascontiguousarray(x[b].T).reshape(NCTILE, 128, T)
        cols = []
        for part in range(3):                          # q, k, v column groups
            base = part * C + g * HPG * D
            cols.append(w_qkv[:, base:base + HPG * D])
        wq = np.ascontiguousarray(np.concatenate(cols, axis=1))
        wpj = np.ascontiguousarray(w_proj[g * HPG * D:(g + 1) * HPG * D, :])
        in_maps.append({
            "xT": xT, "wqkv": wq, "wproj": wpj,
            "cosd": cos_d, "sind": sin_d, "trimask": tri,
        })
    return in_maps


def kernel(x, w_qkv, w_proj):
    nc = _get_program()
    in_maps = _host_prep(x, w_qkv, w_proj)
    res = run_bass_kernel_spmd(nc, in_maps, list(range(8)))

    out = np.empty((B, T, C), dtype=np.float32)
    k = np.empty((B, N_HEAD, T, D), dtype=np.float32)
    v = np.empty((B, N_HEAD, T, D), dtype=np.float32)
    for b in range(B):
        r0 = res.results[2 * b]
        r1 = res.results[2 * b + 1]
        out[b] = r0["out_part"] + r1["out_part"]
        for g, r in ((0, r0), (1, r1)):
            hs = slice(g * HPG, (g + 1) * HPG)
            k[b, hs] = np.transpose(r["k_out"], (0, 2, 1))
            v[b, hs] = np.transpose(r["v_out"], (0, 2, 1))
    return out, k, v


# revision 14
# speedup vs baseline: 1.4488x; 1.0680x over previous
"""Causal self-attention (QKV proj + RoPE + causal softmax attention + out proj)
for Trainium2, distributed over 8 NeuronCores.

Sharding: 4 batches x 2 head-groups (tensor parallel over heads within a batch).
Each core computes, for its (batch b, head-group g of 8 heads):
  - qkv = x[b] @ w_qkv[:, cols(g)]   (feature-major via lhsT = w chunks)
  - RoPE on q, k
  - causal softmax attention for its 8 heads (transposed-scores layout)
  - out_partial = att @ w_proj[rows(g), :]
Host gathers: out[b] = partial(b,0) + partial(b,1)  (the 2-way TP all-reduce),
and reassembles k, v from per-core feature-major slices.

All matmuls run in float32r (fp32 data, hardware rounds operands to 11 mantissa
bits, ~bf16 throughput at N>=512).
"""

import math

import numpy as np

import concourse.bass as bass
import concourse.tile as tile
import concourse.mybir as mybir
from concourse import bacc
from concourse.bass_utils import run_bass_kernel_spmd

F32 = mybir.dt.float32
F32R = mybir.dt.float32r
BF16 = mybir.dt.bfloat16
AF = mybir.ActivationFunctionType

B, T, C = 4, 2048, 2048
N_HEAD = 16
D = C // N_HEAD          # 128
HALF = D // 2            # 64
HPG = 8                  # heads per group (2 groups)
ROPE_BASE = 10000.0
ISQRT_D = 1.0 / math.sqrt(D)

NCTILE = C // 128        # 16 contraction tiles
NQB = T // 512           # 4 query blocks of 512
NKB = T // 128           # 16 key blocks of 128


def _build_program():
    nc = bacc.Bacc("TRN2", target_bir_lowering=False, debug=False, num_devices=8)

    # per-core inputs
    xT_d = nc.dram_tensor("xT", [NCTILE, 128, T], F32R, kind="ExternalInput").ap()
    wqkv_d = nc.dram_tensor("wqkv", [C, 3 * HPG * D], F32R, kind="ExternalInput").ap()
    wproj_d = nc.dram_tensor("wproj", [HPG * D, C], F32R, kind="ExternalInput").ap()
    ccat_d = nc.dram_tensor("ccat", [128, T], F32, kind="ExternalInput").ap()
    scat_d = nc.dram_tensor("scat", [128, T], F32, kind="ExternalInput").ap()
    mask_d = nc.dram_tensor("trimask", [128, 128], BF16, kind="ExternalInput").ap()

    # per-core outputs
    out_d = nc.dram_tensor("out_part", [T, C], F32, kind="ExternalOutput").ap()
    k_out_d = nc.dram_tensor("k_out", [HPG, D, T], F32, kind="ExternalOutput").ap()
    v_out_d = nc.dram_tensor("v_out", [HPG, D, T], F32, kind="ExternalOutput").ap()

    wqkv_r = wqkv_d.rearrange("(c p) f -> p c f", p=128)    # [128, 16, 3072]
    xT_r = xT_d.rearrange("c p t -> p c t")                 # [128, 16, 2048]
    wproj_r = wproj_d.rearrange("(h p) c -> p h c", p=128)  # [128, 8, 2048]

    with tile.TileContext(nc) as tc:
        with tc.tile_pool(name="dram", bufs=1, space="DRAM") as dpool, \
             tc.tile_pool(name="const", bufs=1) as constp:

            q_scr = dpool.tile([HPG, 128, T], F32R, tag="q_scr")
            k_scr = dpool.tile([HPG, 128, T], F32R, tag="k_scr")
            v_scr = dpool.tile([HPG, 128, T], F32R, tag="v_scr")

            mask_sb = constp.tile([128, 128], BF16, tag="mask")
            nc.sync.dma_start(out=mask_sb[:], in_=mask_d)
            ones128_f = constp.tile([128, 1], F32, tag="ones128f")
            nc.vector.memset(ones128_f[:], 1.0)
            ones128 = constp.tile([128, 1], F32R, tag="ones128")
            nc.vector.tensor_copy(ones128[:], ones128_f[:])
            ones1_f = constp.tile([1, 128], F32, tag="ones1f")
            nc.vector.memset(ones1_f[:], 1.0)
            ones1 = constp.tile([1, 128], F32R, tag="ones1")
            nc.vector.tensor_copy(ones1[:], ones1_f[:])
            ident_f = constp.tile([128, 128], F32, tag="ident_f")
            from concourse.masks import make_identity
            make_identity(nc, ident_f[:])
            ident = constp.tile([128, 128], F32R, tag="ident")
            nc.vector.tensor_copy(ident[:], ident_f[:])

            # ---------------- Phase 1: QKV projection + RoPE ----------------
            with tc.tile_pool(name="xh", bufs=1) as xp, \
                 tc.tile_pool(name="wq", bufs=3) as wp, \
                 tc.tile_pool(name="qkvps", bufs=8, space="PSUM") as qkvps, \
                 tc.tile_pool(name="stag", bufs=2) as stp, \
                 tc.tile_pool(name="rope", bufs=2) as rp, \
                 tc.tile_pool(name="cs", bufs=1) as csp:

                # stacked rope tables: ccat = [cos; cos], scat = [-sin; sin]
                ccat_sb = csp.tile([128, T], F32, tag="ccat")
                nc.sync.dma_start(out=ccat_sb[:], in_=ccat_d)
                scat_sb = csp.tile([128, T], F32, tag="scat")
                nc.sync.dma_start(out=scat_sb[:], in_=scat_d)

                xh = xp.tile([128, NCTILE, T], F32R, tag="xh")
                nc.sync.dma_start(out=xh[:], in_=xT_r)

                def rope(ps, tb, out_tile):
                    # out = ps*ccat + rot(ps)*scat, rot = [x2; x1], scat = [-s; s]
                    ts_ = slice(tb * 512, (tb + 1) * 512)
                    rot = rp.tile([128, 512], F32, tag="rot")
                    nc.vector.tensor_copy(rot[0:HALF, :], ps[HALF:128, :])
                    nc.vector.tensor_copy(rot[HALF:128, :], ps[0:HALF, :])
                    p1 = rp.tile([128, 512], F32, tag="p1")
                    nc.vector.tensor_mul(p1[:], ps[:], ccat_sb[:, ts_])
                    p2 = rp.tile([128, 512], F32, tag="p2")
                    nc.vector.tensor_mul(p2[:], rot[:], scat_sb[:, ts_])
                    nc.vector.tensor_add(out_tile[:], p1[:], p2[:])

                for fb in range(3 * HPG):
                    w_t = wp.tile([128, NCTILE, 128], F32R, tag="w")
                    nc.sync.dma_start(
                        out=w_t[:], in_=wqkv_r[:, :, fb * 128:(fb + 1) * 128])
                    pss = [qkvps.tile([128, 512], F32, tag="qkvps",
                                      name=f"qkvps_{fb}_{i}")
                           for i in range(NQB)]
                    for ci in range(NCTILE):
                        for tb in range(NQB):
                            nc.tensor.matmul(
                                pss[tb][:], w_t[:, ci, :],
                                xh[:, ci, tb * 512:(tb + 1) * 512],
                                start=(ci == 0), stop=(ci == NCTILE - 1))
                    for tb in range(NQB):
                        ps = pss[tb]
                        ts_ = slice(tb * 512, (tb + 1) * 512)
                        if fb < HPG:           # q
                            h = fb
                            qf = stp.tile([128, 512], F32R, tag="qstag")
                            rope(ps, tb, qf)
                            nc.sync.dma_start(out=q_scr[h, :, ts_], in_=qf[:])
                        elif fb < 2 * HPG:     # k
                            h = fb - HPG
                            kf = stp.tile([128, 512], F32, tag="kstag")
                            rope(ps, tb, kf)
                            nc.sync.dma_start(out=k_out_d[h, :, ts_], in_=kf[:])
                            nc.sync.dma_start(out=k_scr[h, :, ts_],
                                              in_=kf[:].bitcast(F32R))
                        else:                  # v
                            h = fb - 2 * HPG
                            vf = stp.tile([128, 512], F32, tag="vstag")
                            nc.vector.tensor_copy(vf[:], ps[:])
                            nc.sync.dma_start(out=v_out_d[h, :, ts_], in_=vf[:])
                            nc.sync.dma_start(out=v_scr[h, :, ts_],
                                              in_=vf[:].bitcast(F32R))

            # ---------------- Phase 2+3: attention, then projection ----------
            with tc.tile_pool(name="att", bufs=1) as attp:
                att_all = attp.tile([128, HPG, T], F32R, tag="att_all")

                with tc.tile_pool(name="qh", bufs=2) as qhp, \
                     tc.tile_pool(name="kh", bufs=2) as khp, \
                     tc.tile_pool(name="vh", bufs=2) as vhp, \
                     tc.tile_pool(name="vtok", bufs=2) as vtp, \
                     tc.tile_pool(name="vtps", bufs=1, space="PSUM") as vtps, \
                     tc.tile_pool(name="sps", bufs=3, space="PSUM") as sps, \
                     tc.tile_pool(name="sumps", bufs=1, space="PSUM") as sumps, \
                     tc.tile_pool(name="avps", bufs=2, space="PSUM") as avps, \
                     tc.tile_pool(name="bps", bufs=1, space="PSUM") as bps, \
                     tc.tile_pool(name="probs", bufs=6) as prp, \
                     tc.tile_pool(name="bc", bufs=2) as bcp, \
                     tc.tile_pool(name="rcp", bufs=2) as rcp:

                    for h in range(HPG):
                        q_t = qhp.tile([128, T], F32R, tag="qh")
                        nc.sync.dma_start(out=q_t[:], in_=q_scr[h, :, :])
                        k_t = khp.tile([128, T], F32R, tag="kh")
                        nc.sync.dma_start(out=k_t[:], in_=k_scr[h, :, :])
                        v_h = vhp.tile([128, T], F32R, tag="vh")
                        nc.sync.dma_start(out=v_h[:], in_=v_scr[h, :, :])
                        vtok = vtp.tile([128, NKB, 128], F32R, tag="vtok")
                        for tbk in range(NKB):
                            pvt = vtps.tile([128, 128], F32R, tag="vtps")
                            nc.tensor.transpose(
                                pvt[:], v_h[:, tbk * 128:(tbk + 1) * 128],
                                ident[:])
                            nc.vector.tensor_copy(vtok[:, tbk, :], pvt[:])

                        for qi in range(NQB):
                            nkb = 4 * qi + 4
                            qs = slice(qi * 512, (qi + 1) * 512)
                            ps_sum = sumps.tile([1, 512], F32, tag="ps_sum")
                            ps_o = avps.tile([128, 512], F32, tag="ps_o")
                            pending = []

                            def sum_av(j, pr, left, *, _sum=ps_sum, _o=ps_o,
                                       _nkb=nkb):
                                cs_ = slice(left, 512)
                                nc.tensor.matmul(
                                    _sum[0:1, cs_], ones128[:], pr[:, cs_],
                                    start=(j == 0), stop=(j == _nkb - 1))
                                nc.tensor.matmul(
                                    _o[:, cs_], vtok[:, j, :], pr[:, cs_],
                                    start=(j == 0), stop=(j == _nkb - 1))

                            for kb in range(nkb):
                                ps_s = sps.tile([128, 512], F32, tag="ps_s")
                                nc.tensor.matmul(
                                    ps_s[:], k_t[:, kb * 128:(kb + 1) * 128],
                                    q_t[:, qs], start=True, stop=True)
                                m = kb - 4 * qi
                                left = max(m, 0) * 128
                                pr = prp.tile([128, 512], F32R, tag="probs")
                                nc.scalar.activation(
                                    pr[:, left:], ps_s[:, left:], AF.Exp,
                                    scale=ISQRT_D)
                                if m >= 0:
                                    dg = slice(left, left + 128)
                                    nc.vector.tensor_mul(
                                        pr[:, dg], pr[:, dg], mask_sb[:])
                                pending.append((kb, pr, left))
                                if len(pending) > 2:
                                    sum_av(*pending.pop(0))
                            while pending:
                                sum_av(*pending.pop(0))

                            recip = rcp.tile([1, 512], F32, tag="recip")
                            nc.vector.reciprocal_approx_fast(
                                out=recip[:], in_=ps_sum[:])
                            recipr = rcp.tile([1, 512], F32R, tag="recipr")
                            nc.vector.tensor_copy(recipr[:], recip[:])
                            ps_b = bps.tile([128, 512], F32, tag="ps_b")
                            nc.tensor.matmul(ps_b[:], ones1[:], recipr[:],
                                             start=True, stop=True)
                            bc = bcp.tile([128, 512], F32R, tag="bc")
                            nc.scalar.copy(bc[:], ps_b[:])
                            nc.vector.tensor_mul(att_all[:, h, qs], ps_o[:], bc[:])

                # ---------------- Phase 3: output projection ----------------
                # w_proj fully resident; lhsT (att tile) reused across 4 cb
                with tc.tile_pool(name="wp", bufs=1) as wpp, \
                     tc.tile_pool(name="pps", bufs=8, space="PSUM") as pps, \
                     tc.tile_pool(name="ostag", bufs=4) as osp:
                    wp_t = wpp.tile([128, HPG, C], F32R, tag="wp")
                    nc.sync.dma_start(out=wp_t[:], in_=wproj_r)
                    for tb in range(NKB):
                        psp = [pps.tile([128, 512], F32, tag="ps_p",
                                        name=f"ps_p_{tb}_{i}")
                               for i in range(4)]
                        for hh in range(HPG):
                            for cb in range(4):
                                nc.tensor.matmul(
                                    psp[cb][:],
                                    att_all[:, hh, tb * 128:(tb + 1) * 128],
                                    wp_t[:, hh, cb * 512:(cb + 1) * 512],
                                    start=(hh == 0), stop=(hh == HPG - 1))
                        for cb in range(4):
                            o_sb = osp.tile([128, 512], F32, tag="ostag",
                                            name=f"o_sb_{tb}_{cb}")
                            nc.vector.tensor_copy(o_sb[:], psp[cb][:])
                            nc.sync.dma_start(
                                out=out_d[tb * 128:(tb + 1) * 128,
                                          cb * 512:(cb + 1) * 512],
                                in_=o_sb[:])

    nc.compile()
    return nc


_NC_CACHE = None


def _get_program():
    global _NC_CACHE
    if _NC_CACHE is None:
        _NC_CACHE = _build_program()
    return _NC_CACHE


def _host_prep(x, w_qkv, w_proj):
    """Build the 8 per-core input maps (core = b*2 + g)."""
    x = np.ascontiguousarray(np.asarray(x, dtype=np.float32))
    w_qkv = np.asarray(w_qkv, dtype=np.float32)
    w_proj = np.asarray(w_proj, dtype=np.float32)

    # rope tables, feature-major [HALF, T]
    inv_freq = 1.0 / (ROPE_BASE ** (np.arange(HALF, dtype=np.float32) / HALF))
    pos = np.arange(T, dtype=np.float32)
    freqs = pos[:, None] * inv_freq[None, :]          # [T, HALF]
    cos_t = np.cos(freqs).T.astype(np.float32)      # [HALF, T]
    sin_t = np.sin(freqs).T.astype(np.float32)
    ccat = np.ascontiguousarray(np.concatenate([cos_t, cos_t], axis=0))
    scat = np.ascontiguousarray(np.concatenate([-sin_t, sin_t], axis=0))

    import ml_dtypes
    tri = np.triu(np.ones((128, 128), dtype=np.float32))  # [k,q]: 1 if k<=q
    tri = tri.astype(ml_dtypes.bfloat16)

    in_maps = []
    for core in range(8):
        b, g = divmod(core, 2)
        xT = np.ascontiguousarray(x[b].T).reshape(NCTILE, 128, T)
        cols = []
        for part in range(3):                          # q, k, v column groups
            base = part * C + g * HPG * D
            cols.append(w_qkv[:, base:base + HPG * D])
        wq = np.ascontiguousarray(np.concatenate(cols, axis=1))
        wpj = np.ascontiguousarray(w_proj[g * HPG * D:(g + 1) * HPG * D, :])
        in_maps.append({
            "xT": xT, "wqkv": wq, "wproj": wpj,
            "ccat": ccat, "scat": scat, "trimask": tri,
        })
    return in_maps


def kernel(x, w_qkv, w_proj):
    nc = _get_program()
    in_maps = _host_prep(x, w_qkv, w_proj)
    res = run_bass_kernel_spmd(nc, in_maps, list(range(8)))

    out = np.empty((B, T, C), dtype=np.float32)
    k = np.empty((B, N_HEAD, T, D), dtype=np.float32)
    v = np.empty((B, N_HEAD, T, D), dtype=np.float32)
    for b in range(B):
        r0 = res.results[2 * b]
        r1 = res.results[2 * b + 1]
        out[b] = r0["out_part"] + r1["out_part"]
        for g, r in ((0, r0), (1, r1)):
            hs = slice(g * HPG, (g + 1) * HPG)
            k[b, hs] = np.transpose(r["k_out"], (0, 2, 1))
            v[b, hs] = np.transpose(r["v_out"], (0, 2, 1))
    return out, k, v
